# revision 60
# baseline (speedup 1.0000x reference)
"""Trainium2 Bass kernel for nn_ActorCritic (LSTM with done-resets + heads).

Segment-packed formulation. done ~ Bernoulli(0.5) per (t, env) resets (h, c)
at the START of step t, so the T=512 scan factorizes into ~T*B/2 independent
segments (mean length 2, max ~20). Host-side we split every env's timeline
into segments, deal them across the 8 cores (LPT by positions), and bin-pack
each core's segments into WIDTH=768 column chains of depth L (~22). This is
EXACT for any input (no warmup redundancy): resets inside a chain are handled
by the -30*done f-gate kill (c history) and the m mask (h history), and chain
position 0 gets (h0, c0) for columns seeded with an env's initial segment.
Serial depth drops from T=512 (or 72 for warmup-based time sharding) to ~22.
Columns are sorted by fill so the active set per macro-step is a prefix and
all-pad tail chunks are skipped (the compiled chunk schedule (L, nchs) is
the JIT cache key).

Device per core, per macro-step: up to 3 independent 256-col chunks (the
recurrence chain links chunk (s, j) -> (s+1, j), ~3 list positions apart, so
every engine always has ~2 chunks of unrelated work to hide chain latency):
  - xW GEMM streams 2 chunks ahead into the chunk's [128,1024] PSUM tile
    (2 banks, one start per bank): 8 full-height 128-row pieces, plus the
    38-row tail piece as 2 concurrent row-tile PAIRS (tile_position via
    base_partition 0/64 with host-duplicated data) into different banks;
    4 W_hh matmuls (N=256) accumulate on top.
  - ONE sigmoid over the whole [128,1024] gate tile (slot order [o,i,f,g],
    g block pre-doubled so tanh(g) = 2*sig(2g)-1), ONE tanh per chunk.
  - DVE tail (bf16 2x): t2 = sig_f*c; u = (sig_g'-0.5)*sig_i; c = 2u+t2;
    hs = sig_o*tanh(c) into the bf16 history; hm = hs*m for the recurrence.
  - Heads: wcat-stationary matmul (16-col weight load, hs moving) into a
    [16,512] PSUM bank shared by 2 chunks; fused bias-add + one DMA per
    pair into a transposed [16, NG*256] output (host untransposes).
  - ~4.5us of dummy warmup matmuls flip the PE HAM clock-gate to 8/8 while
    the first x slab lands; input slabs are spread over the gpsimd/sync/
    scalar DMA queues (a single queue tops out at ~90GB/s).

Host-side marshalling (not compute): segment packing, x gather into packed
chunk-list order (bf16, +ones row for bias, +done row for the f-kill), m
mask broadcast, output untranspose + scatter back to (t, env) order.

Measured: ~159-165us HW exec (vs 455us for the 72-step time-sharded
baseline); rel err vs f32 reference ~5.9e-3 (bf16). The chip's P0 power
state adds up to ~+25us when thermally saturated by back-to-back runs.
"""

import heapq
import sys
from contextlib import ExitStack

import numpy as np

sys.path.insert(0, "/opt/trn_rl_repo")

# Problem constants (hardcoded per harness contract).
T = 512
B = 256
NCORES = 8
IN = 292
H = 128
A = 12
NOUT = 13

INA = IN + 2  # +ones row (bias), +done row (f-gate kill)
KSPLITS = [(0, 128), (128, 128)]  # full-height xW k-pieces
K3 = 256  # third piece rows [256:294] run row-paired via tile_position
K3R = INA - K3  # 38
CH = 256  # chunk width (cols)
NJ = 3  # chunks per macro-step
WIDTH = NJ * CH  # column chains per core
GCH = 24  # max g-chunks per input slab (SBUF budget)


def _chunks(NG):
    # small first slabs so the first xW can start ~10us earlier
    sizes = []
    left = NG
    for first in (4, 12):
        n = min(first, left)
        if n:
            sizes.append(n)
        left -= n
    while left > 0:
        n = min(GCH, left)
        sizes.append(n)
        left -= n
    return sizes


def build_nc(key):
    import concourse.bass as bass
    import concourse.tile as tile
    from concourse import bacc, mybir

    f32 = mybir.dt.float32
    bf16 = mybir.dt.bfloat16
    AF = mybir.ActivationFunctionType
    OP = mybir.AluOpType

    L, nchs = key
    # chunk list: per macro-step s, only the first nchs[s] chunks hold live
    # columns (columns are sorted by fill, so the active set is a prefix)
    CHUNKS = [(s, j) for s in range(L) for j in range(nchs[s])]
    QIX = {sj: q for q, sj in enumerate(CHUNKS)}
    NG = len(CHUNKS)
    gchs = _chunks(NG)
    NCH = len(gchs)
    coff = [0]
    for n in gchs:
        coff.append(coff[-1] + n)
    g_slab = []
    for ch, n in enumerate(gchs):
        g_slab += [ch] * n

    nc = bacc.Bacc("TRN2", target_bir_lowering=False, debug=False)

    # ---- I/O (all per-core slices prepared by host) ----
    xt_d = nc.dram_tensor("xt", [INA, NG * CH], bf16, kind="ExternalInput").ap()
    m_d = nc.dram_tensor("m", [128, NG * CH], bf16, kind="ExternalInput").ap()
    h0_d = nc.dram_tensor("h0", [128, WIDTH], bf16, kind="ExternalInput").ap()
    c0_d = nc.dram_tensor("c0", [128, WIDTH], bf16, kind="ExternalInput").ap()
    wih_d = nc.dram_tensor("wih", [INA, 512], bf16, kind="ExternalInput").ap()
    wih3_d = nc.dram_tensor("wih3", [102, 512], bf16, kind="ExternalInput").ap()
    whh_d = nc.dram_tensor("whh", [128, 512], bf16, kind="ExternalInput").ap()
    wcat_d = nc.dram_tensor("wcat", [128, 16], bf16, kind="ExternalInput").ap()
    bhd_d = nc.dram_tensor("bhd", [16, 512], f32, kind="ExternalInput").ap()
    out_d = nc.dram_tensor("out", [16, NG * CH], f32, kind="ExternalOutput").ap()

    with tile.TileContext(nc) as tc, ExitStack() as ctx:
        cst = ctx.enter_context(tc.tile_pool(name="cst", bufs=1))
        big = ctx.enter_context(tc.tile_pool(name="big", bufs=1))
        xtp = ctx.enter_context(tc.tile_pool(name="xtp", bufs=2))
        mp = ctx.enter_context(tc.tile_pool(name="mp", bufs=2))
        wk = ctx.enter_context(tc.tile_pool(name="wk", bufs=3))
        pg_pool = ctx.enter_context(tc.tile_pool(name="pg", bufs=3, space="PSUM"))
        php = ctx.enter_context(tc.tile_pool(name="ph", bufs=2, space="PSUM"))

        # ---- persistent tiles ----
        wih_sb = [cst.tile([sz, 512], bf16, tag=f"wih{k}", name=f"wih{k}")
                  for k, (_, sz) in enumerate(KSPLITS)]
        # k3 piece duplicated at partitions 0:38 and 64:102 so the 4 gate
        # slots run as 2 concurrent row-tile pairs (tile_position via
        # base_partition) into different PSUM banks.
        wih3_sb = cst.tile([102, 512], bf16, tag="wih3", name="wih3")
        whh_sb = cst.tile([128, 512], bf16, tag="whh", name="whh")
        wcat_sb = cst.tile([128, 16], bf16, tag="wcat", name="wcat")
        bhd_sb = cst.tile([16, 512], f32, tag="bhd", name="bhd")
        h0_sb = cst.tile([128, WIDTH], bf16, tag="h0", name="h0")
        c0_sb = cst.tile([128, WIDTH], bf16, tag="c0", name="c0")
        hs_all = big.tile([128, NG * CH], bf16, tag="hs", name="hs")

        # weights on the scalar HWDGE queue so slab0 owns sync/gpsimd.
        # whh FIRST: the PE warmup fillers read it, and the scheduler orders
        # by predicted readiness -- a late whh pushes the fillers after the
        # first real xW where they delay the chain instead of absorbing idle.
        nc.scalar.dma_start(out=whh_sb[:, :], in_=whh_d[:, :])
        for k, (off, sz) in enumerate(KSPLITS):
            nc.scalar.dma_start(out=wih_sb[k][:, :], in_=wih_d[off:off + sz, :])
        nc.scalar.dma_start(out=wih3_sb[:, :], in_=wih3_d[:, :])
        nc.scalar.dma_start(out=wcat_sb[:, :], in_=wcat_d[:, :])
        nc.scalar.dma_start(out=bhd_sb[:, :], in_=bhd_d[:, :])
        nc.scalar.dma_start(out=h0_sb[:, :], in_=h0_d[:, :])
        nc.scalar.dma_start(out=c0_sb[:, :], in_=c0_d[:, :])

        # ---- input slab DMAs ----
        xts = {}
        mts = {}

        def load_slab(ch):
            if ch >= NCH:
                return
            n = gchs[ch] * CH
            c0_, c1_ = coff[ch] * CH, coff[ch + 1] * CH
            h = n // 2
            # spread across the 3 DMA-capable queues: gpsimd (SWDGE),
            # sync (HWDGE), scalar (HWDGE) -- a single queue tops out at
            # ~90GB/s and stalls the PE at slab boundaries.
            tiles = []
            for k, (off, sz) in enumerate(KSPLITS):
                eng = nc.gpsimd if k == 0 else nc.sync
                xt = xtp.tile([sz, GCH * CH], bf16, tag=f"xt{k}", name=f"xt{k}")
                eng.dma_start(out=xt[:, 0:h],
                              in_=xt_d[off:off + sz, c0_:c0_ + h])
                eng.dma_start(out=xt[:, h:n],
                              in_=xt_d[off:off + sz, c0_ + h:c1_])
                tiles.append(xt)
            # k3 rows duplicated at partition bases 0 and 64 (row pairing)
            xt3 = xtp.tile([102, GCH * CH], bf16, tag="xt3", name="xt3")
            for pb in (0, 64):
                nc.gpsimd.dma_start(out=xt3[pb:pb + K3R, 0:h],
                                    in_=xt_d[K3:INA, c0_:c0_ + h])
                nc.gpsimd.dma_start(out=xt3[pb:pb + K3R, h:n],
                                    in_=xt_d[K3:INA, c0_ + h:c1_])
            tiles.append(xt3)
            xts[ch] = tiles
            mt = mp.tile([128, GCH * CH], bf16, tag="mt", name="mt")
            nc.sync.dma_start(out=mt[:, 0:h], in_=m_d[:, c0_:c0_ + h])
            nc.scalar.dma_start(out=mt[:, h:n], in_=m_d[:, c0_ + h:c1_])
            mts[ch] = mt

        load_slab(0)
        load_slab(1)

        # ---- PE warmup + ACT table preload ----
        # HAM starts cold (1.2 GHz) and needs ~3.4us of sustained PE work to
        # un-throttle; dummy matmuls on the (small, early) whh tile warm it
        # while the first x slab DMA is in flight. The sigmoid table load
        # (~2.7us) is also hoisted off the first chunk's critical path.
        scr = php.tile([16, 512], f32, tag="ph", name="scr")
        for i in range(11):
            nc.tensor.matmul(scr[:, 0:256], whh_sb[:, 0:16],
                             whh_sb[0:128, 0:256], start=True, stop=False)
        # sigmoid table preload with a DMA-free input (memset) so the
        # scheduler can run it immediately, not after the weight DMAs
        dumm = wk.tile([128, 16], bf16, tag="dumm", name="dumm")
        nc.gpsimd.memset(dumm[:, :], 0.0)
        dums = wk.tile([128, 16], bf16, tag="dums", name="dums")
        nc.scalar.activation(dums[:, :], dumm[:, :], AF.Sigmoid)

        # ---- xW GEMM straight into the chunk's PSUM tile ----
        # One [128, 4*256] tile per chunk (2 banks; cols = slot*256 + c).
        # Slot order [o, i, f, g]. ONE start per 2KB bank (slot 0 / slot 2
        # first k-piece); later slots' first writes auto-zero, accumulation
        # is per-element via has_written.
        psum_tiles = {}
        PIECES = [(slot, k) for slot in range(4) for k in range(len(KSPLITS))]

        def emit_xw(g):
            if g >= NG:
                return
            pg = pg_pool.tile([128, 1024], f32, tag="pg", name="pg")
            psum_tiles[g] = pg
            tiles = xts[g_slab[g]]
            c0_ = (g - coff[g_slab[g]]) * CH
            for slot, k in PIECES:
                off, sz = KSPLITS[k]
                nc.tensor.matmul(
                    pg[:, slot * CH:slot * CH + CH],
                    wih_sb[k][0:sz, slot * 128:(slot + 1) * 128],
                    tiles[k][0:sz, c0_:c0_ + CH],
                    start=(slot in (0, 2) and k == 0), stop=False)
            # k3 pieces as concurrent row-tile pairs: (slot0@row0, slot2@row64)
            # and (slot1@row0, slot3@row64) target different PSUM banks, so
            # each pair streams in ~256 shared cycles instead of 2x256.
            xt3 = tiles[2]
            for s01 in (0, 1):
                for pb, slot in ((0, s01), (64, s01 + 2)):
                    nc.tensor.matmul(
                        pg[:, slot * CH:slot * CH + CH],
                        wih3_sb[pb:pb + K3R, slot * 128:(slot + 1) * 128],
                        xt3[pb:pb + K3R, c0_:c0_ + CH],
                        start=False, stop=False)

        # heads: wcat stationary (16-col LDW), hs moving; out [16, NG*CH]
        # transposed (host untransposes). 2 chunks share one PSUM bank.
        ph_tiles = {}

        def emit_heads(g):
            pair, loc = g // 2, g % 2
            if loc == 0:
                ph_tiles[pair] = php.tile([16, 512], f32, tag="ph", name="ph")
            ph = ph_tiles[pair]
            nc.tensor.matmul(ph[:, loc * CH:loc * CH + CH],
                             wcat_sb[:, :], hs_all[:, g * CH:g * CH + CH],
                             start=(loc == 0), stop=(loc == 1 or g == NG - 1))

        def flush_heads(pair):
            # fused bias-add + PSUM->SBUF copy, one DMA per chunk pair
            ph = ph_tiles.pop(pair)
            n = min(512, NG * CH - pair * 512)
            ob = wk.tile([16, 512], f32, tag="ob", name="ob")
            nc.vector.scalar_tensor_tensor(
                ob[:, 0:n], ph[:, 0:n], 1.0, bhd_sb[:, 0:n], OP.mult, OP.add)
            nc.scalar.dma_start(out=out_d[:, pair * 512:pair * 512 + n],
                                in_=ob[:, 0:n])

        emit_xw(0)
        emit_xw(1)

        hm_ref = {}  # chain refs keyed by (macro-step, j)
        c_sj = {}
        for j in range(nchs[0]):
            hm_ref[(-1, j)] = h0_sb[:, j * CH:(j + 1) * CH]
            c_sj[(-1, j)] = c0_sb[:, j * CH:(j + 1) * CH]

        sig_ref = {}  # deferred-work refs keyed by list position q
        c_q = {}

        th_ref = {}

        def finish_tanh(qp):
            # deferred tanh(c) of chunk qp -- emitted AFTER this chunk's
            # sigmoid so the chain-critical sigmoid never queues behind it
            if qp < 0 or qp not in c_q:
                return
            th = wk.tile([128, CH], bf16, tag="th", name="th")
            nc.scalar.activation(th[:, :], c_q.pop(qp)[:, :], AF.Tanh)
            th_ref[qp] = th

        def finish_hs(qp):
            # deferred hs/hm of chunk qp -- emitted after the c-chain DVE
            # ops (hm has ~2 chunks of slack before hh consumes it)
            if qp < 0 or qp not in th_ref:
                return
            sp = sig_ref.pop(qp)
            sp_s, sp_j = CHUNKS[qp]
            col = qp * CH
            nc.vector.tensor_mul(hs_all[:, col:col + CH],
                                 sp[:, 0:CH], th_ref.pop(qp)[:, :])
            if (sp_s + 1, sp_j) in QIX:
                hm = wk.tile([128, CH], bf16, tag="hm", name="hm")
                nc.vector.tensor_mul(hm[:, :], hs_all[:, col:col + CH],
                                     mts[g_slab[qp]][
                                         :, (qp - coff[g_slab[qp]])
                                         * CH:(qp + 1 - coff[g_slab[qp]]) * CH])
                hm_ref[(sp_s, sp_j)] = hm

        def finish_chunk(qp):
            finish_tanh(qp)
            finish_hs(qp)

        # ---- the recurrence over the chunk list ----
        for q, (s, j) in enumerate(CHUNKS):
            if q > 0 and q - 1 in coff:
                load_slab(coff.index(q - 1) + 2)

            # tight chain (1-wide tail): the previous list chunk IS the
            # chain parent -- finish it before the hh that consumes its hm
            if (s - 1, j) not in hm_ref:
                finish_chunk(q - 1)

            # PE: W_hh accumulation for chunk q (chain input from (s-1, j))
            pg = psum_tiles.pop(q)
            hm_in = hm_ref.pop((s - 1, j))
            for slot in range(4):
                nc.tensor.matmul(
                    pg[:, slot * CH:slot * CH + CH],
                    whh_sb[:, slot * 128:(slot + 1) * 128],
                    hm_in, start=False, stop=(slot in (1, 3)))

            # PE fillers: xW two chunks ahead, heads two chunks behind
            emit_xw(q + 2)
            if q >= 2:
                emit_heads(q - 2)

            # ACT: the chain-critical sigmoid leads the chunk's ACT block;
            # the previous chunk's tanh (whose input lands late in the
            # previous chunk's DVE tail) runs behind it, not ahead of it.
            sig = wk.tile([128, 1024], bf16, tag="sig", name="sig")
            nc.scalar.activation(sig[:, :], pg[:, :], AF.Sigmoid)
            sig_ref[q] = sig
            finish_tanh(q - 1)

            # DVE: c-chain ops first, then the previous chunk's hs/hm
            t2 = wk.tile([128, CH], bf16, tag="t2", name="t2")
            nc.vector.tensor_mul(t2[:, :], sig[:, 2 * CH:3 * CH],
                                 c_sj.pop((s - 1, j))[:, :])
            u = wk.tile([128, CH], bf16, tag="u", name="u")
            nc.vector.scalar_tensor_tensor(
                u[:, :], sig[:, 3 * CH:4 * CH], 0.5, sig[:, CH:2 * CH],
                OP.subtract, OP.mult)
            cn = wk.tile([128, CH], bf16, tag="cn", name="cn")
            nc.vector.scalar_tensor_tensor(
                cn[:, :], u[:, :], 2.0, t2[:, :], OP.mult, OP.add)
            c_q[q] = cn
            finish_hs(q - 1)
            if (s + 1, j) in QIX:
                c_sj[(s, j)] = cn
            if q >= 2 and (q - 2) % 2 == 1:
                flush_heads((q - 2) // 2)

        # ---- drain: last chunk's tanh/hs, remaining heads ----
        finish_chunk(NG - 1)
        for q in (NG - 2, NG - 1):
            emit_heads(q)
        for pair in sorted(ph_tiles.keys()):
            flush_heads(pair)

    nc.compile()
    return nc


_NC = {}


def _get_nc(key):
    if key not in _NC:
        _NC[key] = build_nc(key)
    return _NC[key]


def _segments(done):
    """Split every env's timeline at done=1 into segments, deal segments
    across cores, pack each core's segments into WIDTH column chains.
    Returns (L, plan): L = max chain depth; plan[c] = per-core slot maps."""
    done = np.asarray(done, dtype=np.int32)
    segs = []  # (length, t0, b, initial)
    for b in range(B):
        col = done[:, b]
        starts = np.flatnonzero(col == 1)
        if len(starts) == 0 or starts[0] != 0:
            starts = np.r_[0, starts]
        lens = np.diff(np.r_[starts, T])
        for t0, ln in zip(starts.tolist(), lens.tolist()):
            segs.append((int(ln), int(t0), b, t0 == 0 and col[0] == 0))

    segs.sort(key=lambda s: (-s[0], s[1], s[2]))
    init_segs = [s for s in segs if s[3]]
    rest_segs = [s for s in segs if not s[3]]

    # deal across cores: initial segments round-robin, rest LPT by positions
    core_segs = [[] for _ in range(NCORES)]
    core_load = [0] * NCORES
    for i, s in enumerate(init_segs):
        c = i % NCORES
        core_segs[c].append(s)
        core_load[c] += s[0]
    heap = [(core_load[c], c) for c in range(NCORES)]
    heapq.heapify(heap)
    for s in rest_segs:
        load, c = heapq.heappop(heap)
        core_segs[c].append(s)
        heapq.heappush(heap, (load + s[0], c))

    # pack each core's segments into WIDTH columns (initial segs first, one
    # per column at position 0; then LPT over all columns)
    plan = []
    Lmax = 0
    for c in range(NCORES):
        ini = [s for s in core_segs[c] if s[3]]
        oth = [s for s in core_segs[c] if not s[3]]
        oth.sort(key=lambda s: (-s[0], s[1], s[2]))
        cols = [[] for _ in range(WIDTH)]
        fill = [0] * WIDTH
        for i, s in enumerate(ini):
            cols[i].append(s)
            fill[i] = s[0]
        heap = [(fill[w], w) for w in range(WIDTH)]
        heapq.heapify(heap)
        for s in oth:
            f, w = heapq.heappop(heap)
            cols[w].append(s)
            heapq.heappush(heap, (f + s[0], w))
        Lc = max(sum(s[0] for s in cols[w]) for w in range(WIDTH))
        Lmax = max(Lmax, Lc)
        plan.append({"cols": cols, "n_init": len(ini)})
    L = Lmax

    # build slot maps (columns sorted by fill desc so the active set at
    # each macro-step is a prefix -> all-pad tail chunks can be skipped)
    acts = np.zeros((NCORES, L), dtype=np.int64)
    for c in range(NCORES):
        cols = plan[c]["cols"]
        fills = np.array([sum(s[0] for s in cw) for cw in cols])
        order = np.argsort(-fills, kind="stable")
        cols = [cols[w] for w in order]
        src = np.full((L, WIDTH), -1, dtype=np.int64)
        de = np.ones((L, WIDTH), dtype=np.float32)  # done row (pads -> 1)
        m = np.zeros((L, WIDTH), dtype=np.float32)
        h0b = np.full(WIDTH, -1, dtype=np.int64)  # env idx for init state
        for w in range(WIDTH):
            s_off = 0
            for (ln, t0, b, initial) in cols[w]:
                ts = np.arange(t0, t0 + ln)
                src[s_off:s_off + ln, w] = ts * B + b
                de[s_off, w] = 0.0 if initial else 1.0
                de[s_off + 1:s_off + ln, w] = 0.0
                m[s_off:s_off + ln - 1, w] = 1.0
                if initial:
                    h0b[w] = b
                s_off += ln
        acts[c] = (np.sort(fills)[::-1][None, :]
                   > np.arange(L)[:, None]).sum(axis=1)
        plan[c] = {"src": src, "de": de, "m": m, "h0b": h0b}

    nchs = tuple(int(-(-int(acts[:, s].max()) // CH)) for s in range(L))
    chunks = [(s, j) for s in range(L) for j in range(nchs[s])]
    # flatten per-core slot maps into chunk-list order
    for c in range(NCORES):
        p = plan[c]
        NGq = len(chunks)
        srcf = np.full((NGq, CH), -1, dtype=np.int64)
        def_ = np.ones((NGq, CH), dtype=np.float32)
        mf = np.zeros((NGq, CH), dtype=np.float32)
        for q, (s, j) in enumerate(chunks):
            srcf[q] = p["src"][s, j * CH:(j + 1) * CH]
            def_[q] = p["de"][s, j * CH:(j + 1) * CH]
            mf[q] = p["m"][s, j * CH:(j + 1) * CH]
        plan[c] = {"src": srcf, "de": def_, "m": mf, "h0b": p["h0b"]}
    return (L, nchs), plan


def _make_in_maps(inputs, L, plan):
    import ml_dtypes

    bf16 = ml_dtypes.bfloat16
    x = np.asarray(inputs["x"], dtype=np.float32)
    done = np.asarray(inputs["done"], dtype=np.int32)
    h0 = np.asarray(inputs["h0"], dtype=np.float32).reshape(B, H)
    c0 = np.asarray(inputs["c0"], dtype=np.float32).reshape(B, H)
    Wih = np.asarray(inputs["W_ih"], dtype=np.float32)
    Whh = np.asarray(inputs["W_hh"], dtype=np.float32)
    bias = (np.asarray(inputs["b_ih"], dtype=np.float32)
            + np.asarray(inputs["b_hh"], dtype=np.float32)).reshape(4 * H)
    Wpi = np.asarray(inputs["W_pi"], dtype=np.float32)
    bpi = np.asarray(inputs["b_pi"], dtype=np.float32).reshape(A)
    Wv = np.asarray(inputs["W_v"], dtype=np.float32)
    bv = np.asarray(inputs["b_v"], dtype=np.float32).reshape(1)

    # gate order i,f,g,o -> o,i,f,g; g block (weights + bias) pre-doubled
    order = np.r_[384:512, 0:128, 128:256, 256:384]
    GS = 384  # g block offset after reorder
    FS = 256  # f block offset after reorder
    WihR = Wih[order].copy()
    WihR[GS:GS + 128] *= 2.0
    WhhR = Whh[order].copy()
    WhhR[GS:GS + 128] *= 2.0
    biasR = bias[order].copy()
    biasR[GS:GS + 128] *= 2.0

    wih_aug = np.zeros((INA, 512), dtype=np.float32)
    wih_aug[0:IN] = WihR.T
    wih_aug[IN] = biasR
    wih_aug[IN + 1, FS:FS + 128] = -30.0  # done kills the f gate
    wih_bf = wih_aug.astype(bf16)
    wih3 = np.zeros((102, 512), dtype=np.float32)
    wih3[0:K3R] = wih_aug[K3:INA]  # duplicated at partition bases 0 and 64
    wih3[64:64 + K3R] = wih_aug[K3:INA]
    wih3_bf = wih3.astype(bf16)
    whh_bf = np.ascontiguousarray(WhhR.T).astype(bf16)

    wcat = np.zeros((128, 16), dtype=np.float32)
    wcat[:, 0:A] = Wpi.T
    wcat[:, A] = Wv[0]
    wcat_bf = wcat.astype(bf16)
    bgrp = np.zeros(16, dtype=np.float32)
    bgrp[0:A] = bpi
    bgrp[A] = bv[0]
    bhd = np.ascontiguousarray(
        np.broadcast_to(bgrp[:, None], (16, 512)))  # [16, 512]

    xT = np.ascontiguousarray(x.transpose(2, 0, 1).reshape(IN, T * B))
    h0T = h0.T  # [H, B]
    c0T = c0.T

    in_maps = []
    for c in range(NCORES):
        p = plan[c]
        src = p["src"].reshape(-1)  # [NG*CH] in chunk-list order
        ncol = src.size
        valid = src >= 0
        xt = np.zeros((INA, ncol), dtype=np.float32)
        xt[0:IN, valid] = xT[:, src[valid]]
        xt[IN] = 1.0
        xt[IN + 1] = p["de"].reshape(-1)

        m_bc = np.ascontiguousarray(np.broadcast_to(
            p["m"].reshape(1, ncol), (128, ncol))).astype(bf16)

        h0c = np.zeros((H, WIDTH), dtype=np.float32)
        c0c = np.zeros((H, WIDTH), dtype=np.float32)
        wsel = p["h0b"] >= 0
        h0c[:, wsel] = h0T[:, p["h0b"][wsel]]
        c0c[:, wsel] = c0T[:, p["h0b"][wsel]]

        in_maps.append({
            "xt": xt.astype(bf16),
            "m": m_bc,
            "h0": h0c.astype(bf16),
            "c0": c0c.astype(bf16),
            "wih": wih_bf,
            "wih3": wih3_bf,
            "whh": whh_bf,
            "wcat": wcat_bf,
            "bhd": bhd,
        })
    return in_maps


def _try_device_reset():
    try:
        import ctypes

        import jax

        jax.devices()
        lib = ctypes.CDLL("/opt/axon/libaxon_pjrt.so")
        if hasattr(lib, "axon_reset"):
            lib.axon_reset.restype = ctypes.c_int64
            lib.axon_reset()
    except Exception:
        pass


def kernel(**inputs):
    from concourse.bass_utils import run_bass_kernel_spmd

    done = np.asarray(inputs["done"], dtype=np.int32)
    L, plan = _segments(done)
    nc = _get_nc(L)
    in_maps = _make_in_maps(inputs, L, plan)
    try:
        res = run_bass_kernel_spmd(nc, in_maps, core_ids=list(range(NCORES)))
    except Exception:
        _try_device_reset()
        res = run_bass_kernel_spmd(nc, in_maps, core_ids=list(range(NCORES)))
    full = np.empty((T * B, NOUT), dtype=np.float32)
    for c in range(NCORES):
        out = np.ascontiguousarray(res.results[c]["out"].T)  # [NG*CH, 16]
        src = plan[c]["src"].reshape(-1)
        valid = src >= 0
        full[src[valid]] = out[valid][:, 0:NOUT]
    return full


# revision 61
# speedup vs baseline: 1.0207x; 1.0207x over previous
"""Trainium2 Bass kernel for nn_ActorCritic (LSTM with done-resets + heads).

Segment-packed formulation. done ~ Bernoulli(0.5) per (t, env) resets (h, c)
at the START of step t, so the T=512 scan factorizes into ~T*B/2 independent
segments (mean length 2, max ~20). Host-side we split every env's timeline
into segments, deal them across the 8 cores (LPT by positions), and bin-pack
each core's segments into WIDTH=768 column chains of depth L (~22). This is
EXACT for any input (no warmup redundancy): resets inside a chain are handled
by the -30*done f-gate kill (c history) and the m mask (h history), and chain
position 0 gets (h0, c0) for columns seeded with an env's initial segment.
Serial depth drops from T=512 (or 72 for warmup-based time sharding) to ~22.
Columns are sorted by fill so the active set per macro-step is a prefix and
all-pad tail chunks are skipped (the compiled chunk schedule (L, nchs) is
the JIT cache key).

Device per core, per macro-step: up to 3 independent 256-col chunks (the
recurrence chain links chunk (s, j) -> (s+1, j), ~3 list positions apart, so
every engine always has ~2 chunks of unrelated work to hide chain latency):
  - xW GEMM streams 2 chunks ahead into the chunk's [128,1024] PSUM tile
    (2 banks, one start per bank): 8 full-height 128-row pieces, plus the
    38-row tail piece as 2 concurrent row-tile PAIRS (tile_position via
    base_partition 0/64 with host-duplicated data) into different banks;
    4 W_hh matmuls (N=256) accumulate on top.
  - ONE sigmoid over the whole [128,1024] gate tile (slot order [o,i,f,g],
    g block pre-doubled so tanh(g) = 2*sig(2g)-1), ONE tanh per chunk.
  - DVE tail (bf16 2x): t2 = sig_f*c; u = (sig_g'-0.5)*sig_i; c = 2u+t2;
    hs = sig_o*tanh(c) into the bf16 history; hm = hs*m for the recurrence.
  - Heads: wcat-stationary matmul (16-col weight load, hs moving) into a
    [16,512] PSUM bank shared by 2 chunks; fused bias-add + one DMA per
    pair into a transposed [16, NG*256] output (host untransposes).
  - ~4.5us of dummy warmup matmuls flip the PE HAM clock-gate to 8/8 while
    the first x slab lands; input slabs are spread over the gpsimd/sync/
    scalar DMA queues (a single queue tops out at ~90GB/s).

Host-side marshalling (not compute): segment packing, x gather into packed
chunk-list order (bf16, +ones row for bias, +done row for the f-kill), m
mask broadcast, output untranspose + scatter back to (t, env) order.

Measured: ~159-165us HW exec (vs 455us for the 72-step time-sharded
baseline); rel err vs f32 reference ~5.9e-3 (bf16). The chip's P0 power
state adds up to ~+25us when thermally saturated by back-to-back runs.
"""

import heapq
import sys
from contextlib import ExitStack

import numpy as np

sys.path.insert(0, "/opt/trn_rl_repo")

# Problem constants (hardcoded per harness contract).
T = 512
B = 256
NCORES = 8
IN = 292
H = 128
A = 12
NOUT = 13

INA = IN + 2  # +ones row (bias), +done row (f-gate kill)
KSPLITS = [(0, 128), (128, 128)]  # full-height xW k-pieces
K3 = 256  # third piece rows [256:294] run row-paired via tile_position
K3R = INA - K3  # 38
CH = 256  # chunk width (cols)
NJ = 3  # chunks per macro-step
WIDTH = NJ * CH  # column chains per core
GCH = 24  # max g-chunks per input slab (SBUF budget)


def _chunks(NG):
    # small first slabs so the first xW can start ~10us earlier
    sizes = []
    left = NG
    for first in (4, 12):
        n = min(first, left)
        if n:
            sizes.append(n)
        left -= n
    while left > 0:
        n = min(GCH, left)
        sizes.append(n)
        left -= n
    return sizes


def build_nc(key):
    import concourse.bass as bass
    import concourse.tile as tile
    from concourse import bacc, mybir

    f32 = mybir.dt.float32
    bf16 = mybir.dt.bfloat16
    AF = mybir.ActivationFunctionType
    OP = mybir.AluOpType

    L, nchs = key
    # chunk list: per macro-step s, only the first nchs[s] chunks hold live
    # columns (columns are sorted by fill, so the active set is a prefix)
    CHUNKS = [(s, j) for s in range(L) for j in range(nchs[s])]
    QIX = {sj: q for q, sj in enumerate(CHUNKS)}
    NG = len(CHUNKS)
    gchs = _chunks(NG)
    NCH = len(gchs)
    coff = [0]
    for n in gchs:
        coff.append(coff[-1] + n)
    g_slab = []
    for ch, n in enumerate(gchs):
        g_slab += [ch] * n

    nc = bacc.Bacc("TRN2", target_bir_lowering=False, debug=False)

    # ---- I/O (all per-core slices prepared by host) ----
    xt_d = nc.dram_tensor("xt", [INA, NG * CH], bf16, kind="ExternalInput").ap()
    m_d = nc.dram_tensor("m", [128, NG * CH], bf16, kind="ExternalInput").ap()
    h0_d = nc.dram_tensor("h0", [128, WIDTH], bf16, kind="ExternalInput").ap()
    c0_d = nc.dram_tensor("c0", [128, WIDTH], bf16, kind="ExternalInput").ap()
    wih_d = nc.dram_tensor("wih", [INA, 512], bf16, kind="ExternalInput").ap()
    wih3_d = nc.dram_tensor("wih3", [102, 512], bf16, kind="ExternalInput").ap()
    whh_d = nc.dram_tensor("whh", [128, 512], bf16, kind="ExternalInput").ap()
    wcat_d = nc.dram_tensor("wcat", [128, 16], bf16, kind="ExternalInput").ap()
    bhd_d = nc.dram_tensor("bhd", [16, 512], f32, kind="ExternalInput").ap()
    out_d = nc.dram_tensor("out", [16, NG * CH], f32, kind="ExternalOutput").ap()

    with tile.TileContext(nc) as tc, ExitStack() as ctx:
        cst = ctx.enter_context(tc.tile_pool(name="cst", bufs=1))
        big = ctx.enter_context(tc.tile_pool(name="big", bufs=1))
        xtp = ctx.enter_context(tc.tile_pool(name="xtp", bufs=2))
        mp = ctx.enter_context(tc.tile_pool(name="mp", bufs=2))
        wk = ctx.enter_context(tc.tile_pool(name="wk", bufs=3))
        pg_pool = ctx.enter_context(tc.tile_pool(name="pg", bufs=3, space="PSUM"))
        php = ctx.enter_context(tc.tile_pool(name="ph", bufs=2, space="PSUM"))

        # ---- persistent tiles ----
        wih_sb = [cst.tile([sz, 512], bf16, tag=f"wih{k}", name=f"wih{k}")
                  for k, (_, sz) in enumerate(KSPLITS)]
        # k3 piece duplicated at partitions 0:38 and 64:102 so the 4 gate
        # slots run as 2 concurrent row-tile pairs (tile_position via
        # base_partition) into different PSUM banks.
        wih3_sb = cst.tile([102, 512], bf16, tag="wih3", name="wih3")
        whh_sb = cst.tile([128, 512], bf16, tag="whh", name="whh")
        wcat_sb = cst.tile([128, 16], bf16, tag="wcat", name="wcat")
        bhd_sb = cst.tile([16, 512], f32, tag="bhd", name="bhd")
        h0_sb = cst.tile([128, WIDTH], bf16, tag="h0", name="h0")
        c0_sb = cst.tile([128, WIDTH], bf16, tag="c0", name="c0")
        hs_all = big.tile([128, NG * CH], bf16, tag="hs", name="hs")

        # weights on the scalar HWDGE queue so slab0 owns sync/gpsimd.
        # whh FIRST: the PE warmup fillers read it, and the scheduler orders
        # by predicted readiness -- a late whh pushes the fillers after the
        # first real xW where they delay the chain instead of absorbing idle.
        nc.scalar.dma_start(out=whh_sb[:, :], in_=whh_d[:, :])
        for k, (off, sz) in enumerate(KSPLITS):
            nc.scalar.dma_start(out=wih_sb[k][:, :], in_=wih_d[off:off + sz, :])
        nc.scalar.dma_start(out=wih3_sb[:, :], in_=wih3_d[:, :])
        nc.scalar.dma_start(out=wcat_sb[:, :], in_=wcat_d[:, :])
        nc.scalar.dma_start(out=bhd_sb[:, :], in_=bhd_d[:, :])
        nc.scalar.dma_start(out=h0_sb[:, :], in_=h0_d[:, :])
        nc.scalar.dma_start(out=c0_sb[:, :], in_=c0_d[:, :])

        # ---- input slab DMAs ----
        xts = {}
        mts = {}

        def load_slab(ch):
            if ch >= NCH:
                return
            n = gchs[ch] * CH
            c0_, c1_ = coff[ch] * CH, coff[ch + 1] * CH
            h = n // 2
            # spread across the 3 DMA-capable queues: gpsimd (SWDGE),
            # sync (HWDGE), scalar (HWDGE) -- a single queue tops out at
            # ~90GB/s and stalls the PE at slab boundaries.
            tiles = []
            for k, (off, sz) in enumerate(KSPLITS):
                eng = nc.gpsimd if k == 0 else nc.sync
                xt = xtp.tile([sz, GCH * CH], bf16, tag=f"xt{k}", name=f"xt{k}")
                eng.dma_start(out=xt[:, 0:h],
                              in_=xt_d[off:off + sz, c0_:c0_ + h])
                eng.dma_start(out=xt[:, h:n],
                              in_=xt_d[off:off + sz, c0_ + h:c1_])
                tiles.append(xt)
            # k3 rows duplicated at partition bases 0 and 64 (row pairing)
            xt3 = xtp.tile([102, GCH * CH], bf16, tag="xt3", name="xt3")
            for pb in (0, 64):
                nc.gpsimd.dma_start(out=xt3[pb:pb + K3R, 0:h],
                                    in_=xt_d[K3:INA, c0_:c0_ + h])
                nc.gpsimd.dma_start(out=xt3[pb:pb + K3R, h:n],
                                    in_=xt_d[K3:INA, c0_ + h:c1_])
            tiles.append(xt3)
            xts[ch] = tiles
            mt = mp.tile([128, GCH * CH], bf16, tag="mt", name="mt")
            nc.sync.dma_start(out=mt[:, 0:h], in_=m_d[:, c0_:c0_ + h])
            nc.scalar.dma_start(out=mt[:, h:n], in_=m_d[:, c0_ + h:c1_])
            mts[ch] = mt

        load_slab(0)
        load_slab(1)

        # ---- PE warmup + ACT table preload ----
        # HAM starts cold (1.2 GHz) and needs ~3.4us of sustained PE work to
        # un-throttle; dummy matmuls on the (small, early) whh tile warm it
        # while the first x slab DMA is in flight. The sigmoid table load
        # (~2.7us) is also hoisted off the first chunk's critical path.
        scr = php.tile([16, 512], f32, tag="ph", name="scr")
        for i in range(11):
            nc.tensor.matmul(scr[:, 0:256], whh_sb[:, 0:16],
                             whh_sb[0:128, 0:256], start=True, stop=False)
        # sigmoid table preload with a DMA-free input (memset) so the
        # scheduler can run it immediately, not after the weight DMAs
        dumm = wk.tile([128, 16], bf16, tag="dumm", name="dumm")
        nc.gpsimd.memset(dumm[:, :], 0.0)
        dums = wk.tile([128, 16], bf16, tag="dums", name="dums")
        nc.scalar.activation(dums[:, :], dumm[:, :], AF.Sigmoid)

        # ---- xW GEMM straight into the chunk's PSUM tile ----
        # One [128, 4*256] tile per chunk (2 banks; cols = slot*256 + c).
        # Slot order [o, i, f, g]. ONE start per 2KB bank (slot 0 / slot 2
        # first k-piece); later slots' first writes auto-zero, accumulation
        # is per-element via has_written.
        psum_tiles = {}
        PIECES = [(slot, k) for slot in range(4) for k in range(len(KSPLITS))]

        def emit_xw(g):
            if g >= NG:
                return
            pg = pg_pool.tile([128, 1024], f32, tag="pg", name="pg")
            psum_tiles[g] = pg
            tiles = xts[g_slab[g]]
            c0_ = (g - coff[g_slab[g]]) * CH
            for slot, k in PIECES:
                off, sz = KSPLITS[k]
                nc.tensor.matmul(
                    pg[:, slot * CH:slot * CH + CH],
                    wih_sb[k][0:sz, slot * 128:(slot + 1) * 128],
                    tiles[k][0:sz, c0_:c0_ + CH],
                    start=(slot in (0, 2) and k == 0), stop=False)
            # k3 pieces as concurrent row-tile pairs: (slot0@row0, slot2@row64)
            # and (slot1@row0, slot3@row64) target different PSUM banks, so
            # each pair streams in ~256 shared cycles instead of 2x256.
            xt3 = tiles[2]
            for s01 in (0, 1):
                for pb, slot in ((0, s01), (64, s01 + 2)):
                    nc.tensor.matmul(
                        pg[:, slot * CH:slot * CH + CH],
                        wih3_sb[pb:pb + K3R, slot * 128:(slot + 1) * 128],
                        xt3[pb:pb + K3R, c0_:c0_ + CH],
                        start=False, stop=False)

        # heads: wcat stationary (16-col LDW), hs moving; out [16, NG*CH]
        # transposed (host untransposes). 2 chunks share one PSUM bank.
        ph_tiles = {}

        def emit_heads(g):
            pair, loc = g // 2, g % 2
            if loc == 0:
                ph_tiles[pair] = php.tile([16, 512], f32, tag="ph", name="ph")
            ph = ph_tiles[pair]
            nc.tensor.matmul(ph[:, loc * CH:loc * CH + CH],
                             wcat_sb[:, :], hs_all[:, g * CH:g * CH + CH],
                             start=(loc == 0), stop=(loc == 1 or g == NG - 1))

        def flush_heads(pair):
            # fused bias-add + PSUM->SBUF copy, one DMA per chunk pair
            ph = ph_tiles.pop(pair)
            n = min(512, NG * CH - pair * 512)
            ob = wk.tile([16, 512], f32, tag="ob", name="ob")
            nc.vector.scalar_tensor_tensor(
                ob[:, 0:n], ph[:, 0:n], 1.0, bhd_sb[:, 0:n], OP.mult, OP.add)
            nc.scalar.dma_start(out=out_d[:, pair * 512:pair * 512 + n],
                                in_=ob[:, 0:n])

        emit_xw(0)
        emit_xw(1)

        hm_ref = {}  # chain refs keyed by (macro-step, j)
        c_sj = {}
        for j in range(nchs[0]):
            hm_ref[(-1, j)] = h0_sb[:, j * CH:(j + 1) * CH]
            c_sj[(-1, j)] = c0_sb[:, j * CH:(j + 1) * CH]

        sig_ref = {}  # deferred-work refs keyed by list position q
        c_q = {}

        def finish_chunk(qp):
            # deferred tail of chunk qp: tanh(c), hs into history, masked hm
            if qp < 0 or qp not in sig_ref:
                return
            sp = sig_ref.pop(qp)
            sp_s, sp_j = CHUNKS[qp]
            th = wk.tile([128, CH], bf16, tag="th", name="th")
            nc.scalar.activation(th[:, :], c_q.pop(qp)[:, :], AF.Tanh)
            col = qp * CH
            nc.vector.tensor_mul(hs_all[:, col:col + CH],
                                 sp[:, 0:CH], th[:, :])
            if (sp_s + 1, sp_j) in QIX:
                hm = wk.tile([128, CH], bf16, tag="hm", name="hm")
                nc.vector.tensor_mul(hm[:, :], hs_all[:, col:col + CH],
                                     mts[g_slab[qp]][
                                         :, (qp - coff[g_slab[qp]])
                                         * CH:(qp + 1 - coff[g_slab[qp]]) * CH])
                hm_ref[(sp_s, sp_j)] = hm

        # ---- the recurrence over the chunk list ----
        for q, (s, j) in enumerate(CHUNKS):
            if q > 0 and q - 1 in coff:
                load_slab(coff.index(q - 1) + 2)

            # tight chain (1-wide tail): the previous list chunk IS the
            # chain parent -- finish it before the hh that consumes its hm
            if (s - 1, j) not in hm_ref:
                finish_chunk(q - 1)

            # PE: W_hh accumulation for chunk q (chain input from (s-1, j))
            pg = psum_tiles.pop(q)
            hm_in = hm_ref.pop((s - 1, j))
            for slot in range(4):
                nc.tensor.matmul(
                    pg[:, slot * CH:slot * CH + CH],
                    whh_sb[:, slot * 128:(slot + 1) * 128],
                    hm_in, start=False, stop=(slot in (1, 3)))

            # PE fillers: xW two chunks ahead, heads two chunks behind
            emit_xw(q + 2)
            if q >= 2:
                emit_heads(q - 2)

            # ACT/DVE: deferred tail of the previous chunk, then this
            # chunk's sigmoid and c-path.
            finish_chunk(q - 1)
            sig = wk.tile([128, 1024], bf16, tag="sig", name="sig")
            nc.scalar.activation(sig[:, :], pg[:, :], AF.Sigmoid)
            sig_ref[q] = sig

            t2 = wk.tile([128, CH], bf16, tag="t2", name="t2")
            nc.vector.tensor_mul(t2[:, :], sig[:, 2 * CH:3 * CH],
                                 c_sj.pop((s - 1, j))[:, :])
            u = wk.tile([128, CH], bf16, tag="u", name="u")
            nc.vector.scalar_tensor_tensor(
                u[:, :], sig[:, 3 * CH:4 * CH], 0.5, sig[:, CH:2 * CH],
                OP.subtract, OP.mult)
            cn = wk.tile([128, CH], bf16, tag="cn", name="cn")
            nc.vector.scalar_tensor_tensor(
                cn[:, :], u[:, :], 2.0, t2[:, :], OP.mult, OP.add)
            c_q[q] = cn
            if (s + 1, j) in QIX:
                c_sj[(s, j)] = cn
            if q >= 2 and (q - 2) % 2 == 1:
                flush_heads((q - 2) // 2)

        # ---- drain: last chunk's tanh/hs, remaining heads ----
        finish_chunk(NG - 1)
        for q in (NG - 2, NG - 1):
            emit_heads(q)
        for pair in sorted(ph_tiles.keys()):
            flush_heads(pair)

    nc.compile()
    return nc


_NC = {}


def _get_nc(key):
    if key not in _NC:
        _NC[key] = build_nc(key)
    return _NC[key]


def _segments(done):
    """Split every env's timeline at done=1 into segments, deal segments
    across cores, pack each core's segments into WIDTH column chains.
    Returns (L, plan): L = max chain depth; plan[c] = per-core slot maps."""
    done = np.asarray(done, dtype=np.int32)
    segs = []  # (length, t0, b, initial)
    for b in range(B):
        col = done[:, b]
        starts = np.flatnonzero(col == 1)
        if len(starts) == 0 or starts[0] != 0:
            starts = np.r_[0, starts]
        lens = np.diff(np.r_[starts, T])
        for t0, ln in zip(starts.tolist(), lens.tolist()):
            segs.append((int(ln), int(t0), b, t0 == 0 and col[0] == 0))

    segs.sort(key=lambda s: (-s[0], s[1], s[2]))
    init_segs = [s for s in segs if s[3]]
    rest_segs = [s for s in segs if not s[3]]

    # deal across cores: initial segments round-robin, rest LPT by positions
    core_segs = [[] for _ in range(NCORES)]
    core_load = [0] * NCORES
    for i, s in enumerate(init_segs):
        c = i % NCORES
        core_segs[c].append(s)
        core_load[c] += s[0]
    heap = [(core_load[c], c) for c in range(NCORES)]
    heapq.heapify(heap)
    for s in rest_segs:
        load, c = heapq.heappop(heap)
        core_segs[c].append(s)
        heapq.heappush(heap, (load + s[0], c))

    # pack each core's segments into WIDTH columns (initial segs first, one
    # per column at position 0; then LPT over all columns)
    plan = []
    Lmax = 0
    for c in range(NCORES):
        ini = [s for s in core_segs[c] if s[3]]
        oth = [s for s in core_segs[c] if not s[3]]
        oth.sort(key=lambda s: (-s[0], s[1], s[2]))
        cols = [[] for _ in range(WIDTH)]
        fill = [0] * WIDTH
        for i, s in enumerate(ini):
            cols[i].append(s)
            fill[i] = s[0]
        heap = [(fill[w], w) for w in range(WIDTH)]
        heapq.heapify(heap)
        for s in oth:
            f, w = heapq.heappop(heap)
            cols[w].append(s)
            heapq.heappush(heap, (f + s[0], w))
        Lc = max(sum(s[0] for s in cols[w]) for w in range(WIDTH))
        Lmax = max(Lmax, Lc)
        plan.append({"cols": cols, "n_init": len(ini)})
    L = Lmax

    # build slot maps (columns sorted by fill desc so the active set at
    # each macro-step is a prefix -> all-pad tail chunks can be skipped)
    acts = np.zeros((NCORES, L), dtype=np.int64)
    for c in range(NCORES):
        cols = plan[c]["cols"]
        fills = np.array([sum(s[0] for s in cw) for cw in cols])
        order = np.argsort(-fills, kind="stable")
        cols = [cols[w] for w in order]
        src = np.full((L, WIDTH), -1, dtype=np.int64)
        de = np.ones((L, WIDTH), dtype=np.float32)  # done row (pads -> 1)
        m = np.zeros((L, WIDTH), dtype=np.float32)
        h0b = np.full(WIDTH, -1, dtype=np.int64)  # env idx for init state
        for w in range(WIDTH):
            s_off = 0
            for (ln, t0, b, initial) in cols[w]:
                ts = np.arange(t0, t0 + ln)
                src[s_off:s_off + ln, w] = ts * B + b
                de[s_off, w] = 0.0 if initial else 1.0
                de[s_off + 1:s_off + ln, w] = 0.0
                m[s_off:s_off + ln - 1, w] = 1.0
                if initial:
                    h0b[w] = b
                s_off += ln
        acts[c] = (np.sort(fills)[::-1][None, :]
                   > np.arange(L)[:, None]).sum(axis=1)
        plan[c] = {"src": src, "de": de, "m": m, "h0b": h0b}

    nchs = tuple(int(-(-int(acts[:, s].max()) // CH)) for s in range(L))
    chunks = [(s, j) for s in range(L) for j in range(nchs[s])]
    # flatten per-core slot maps into chunk-list order
    for c in range(NCORES):
        p = plan[c]
        NGq = len(chunks)
        srcf = np.full((NGq, CH), -1, dtype=np.int64)
        def_ = np.ones((NGq, CH), dtype=np.float32)
        mf = np.zeros((NGq, CH), dtype=np.float32)
        for q, (s, j) in enumerate(chunks):
            srcf[q] = p["src"][s, j * CH:(j + 1) * CH]
            def_[q] = p["de"][s, j * CH:(j + 1) * CH]
            mf[q] = p["m"][s, j * CH:(j + 1) * CH]
        plan[c] = {"src": srcf, "de": def_, "m": mf, "h0b": p["h0b"]}
    return (L, nchs), plan


def _make_in_maps(inputs, L, plan):
    import ml_dtypes

    bf16 = ml_dtypes.bfloat16
    x = np.asarray(inputs["x"], dtype=np.float32)
    done = np.asarray(inputs["done"], dtype=np.int32)
    h0 = np.asarray(inputs["h0"], dtype=np.float32).reshape(B, H)
    c0 = np.asarray(inputs["c0"], dtype=np.float32).reshape(B, H)
    Wih = np.asarray(inputs["W_ih"], dtype=np.float32)
    Whh = np.asarray(inputs["W_hh"], dtype=np.float32)
    bias = (np.asarray(inputs["b_ih"], dtype=np.float32)
            + np.asarray(inputs["b_hh"], dtype=np.float32)).reshape(4 * H)
    Wpi = np.asarray(inputs["W_pi"], dtype=np.float32)
    bpi = np.asarray(inputs["b_pi"], dtype=np.float32).reshape(A)
    Wv = np.asarray(inputs["W_v"], dtype=np.float32)
    bv = np.asarray(inputs["b_v"], dtype=np.float32).reshape(1)

    # gate order i,f,g,o -> o,i,f,g; g block (weights + bias) pre-doubled
    order = np.r_[384:512, 0:128, 128:256, 256:384]
    GS = 384  # g block offset after reorder
    FS = 256  # f block offset after reorder
    WihR = Wih[order].copy()
    WihR[GS:GS + 128] *= 2.0
    WhhR = Whh[order].copy()
    WhhR[GS:GS + 128] *= 2.0
    biasR = bias[order].copy()
    biasR[GS:GS + 128] *= 2.0

    wih_aug = np.zeros((INA, 512), dtype=np.float32)
    wih_aug[0:IN] = WihR.T
    wih_aug[IN] = biasR
    wih_aug[IN + 1, FS:FS + 128] = -30.0  # done kills the f gate
    wih_bf = wih_aug.astype(bf16)
    wih3 = np.zeros((102, 512), dtype=np.float32)
    wih3[0:K3R] = wih_aug[K3:INA]  # duplicated at partition bases 0 and 64
    wih3[64:64 + K3R] = wih_aug[K3:INA]
    wih3_bf = wih3.astype(bf16)
    whh_bf = np.ascontiguousarray(WhhR.T).astype(bf16)

    wcat = np.zeros((128, 16), dtype=np.float32)
    wcat[:, 0:A] = Wpi.T
    wcat[:, A] = Wv[0]
    wcat_bf = wcat.astype(bf16)
    bgrp = np.zeros(16, dtype=np.float32)
    bgrp[0:A] = bpi
    bgrp[A] = bv[0]
    bhd = np.ascontiguousarray(
        np.broadcast_to(bgrp[:, None], (16, 512)))  # [16, 512]

    xT = np.ascontiguousarray(x.transpose(2, 0, 1).reshape(IN, T * B))
    h0T = h0.T  # [H, B]
    c0T = c0.T

    in_maps = []
    for c in range(NCORES):
        p = plan[c]
        src = p["src"].reshape(-1)  # [NG*CH] in chunk-list order
        ncol = src.size
        valid = src >= 0
        xt = np.zeros((INA, ncol), dtype=np.float32)
        xt[0:IN, valid] = xT[:, src[valid]]
        xt[IN] = 1.0
        xt[IN + 1] = p["de"].reshape(-1)

        m_bc = np.ascontiguousarray(np.broadcast_to(
            p["m"].reshape(1, ncol), (128, ncol))).astype(bf16)

        h0c = np.zeros((H, WIDTH), dtype=np.float32)
        c0c = np.zeros((H, WIDTH), dtype=np.float32)
        wsel = p["h0b"] >= 0
        h0c[:, wsel] = h0T[:, p["h0b"][wsel]]
        c0c[:, wsel] = c0T[:, p["h0b"][wsel]]

        in_maps.append({
            "xt": xt.astype(bf16),
            "m": m_bc,
            "h0": h0c.astype(bf16),
            "c0": c0c.astype(bf16),
            "wih": wih_bf,
            "wih3": wih3_bf,
            "whh": whh_bf,
            "wcat": wcat_bf,
            "bhd": bhd,
        })
    return in_maps


def _try_device_reset():
    try:
        import ctypes

        import jax

        jax.devices()
        lib = ctypes.CDLL("/opt/axon/libaxon_pjrt.so")
        if hasattr(lib, "axon_reset"):
            lib.axon_reset.restype = ctypes.c_int64
            lib.axon_reset()
    except Exception:
        pass


def kernel(**inputs):
    from concourse.bass_utils import run_bass_kernel_spmd

    done = np.asarray(inputs["done"], dtype=np.int32)
    L, plan = _segments(done)
    nc = _get_nc(L)
    in_maps = _make_in_maps(inputs, L, plan)
    try:
        res = run_bass_kernel_spmd(nc, in_maps, core_ids=list(range(NCORES)))
    except Exception:
        _try_device_reset()
        res = run_bass_kernel_spmd(nc, in_maps, core_ids=list(range(NCORES)))
    full = np.empty((T * B, NOUT), dtype=np.float32)
    for c in range(NCORES):
        out = np.ascontiguousarray(res.results[c]["out"].T)  # [NG*CH, 16]
        src = plan[c]["src"].reshape(-1)
        valid = src >= 0
        full[src[valid]] = out[valid][:, 0:NOUT]
    return full
